# revision 1
# baseline (speedup 1.0000x reference)
"""CAREConv GNN layer on 8 TRN2 NeuronCores.

Algorithm (per edge type, 3 types):
  mlp_h = tanh(feat @ W_mlp + b_mlp)            [N, 2]
  d[n,k] = ||mlp_h[nbr[n,k]] - mlp_h[n]||_1     [N, 32]
  keep 16 smallest-d neighbors, agg = tanh(mean(feat[sel]))
  out = (0.5*(agg0+agg1+agg2) + feat) -> tanh -> @ W_lin + b_lin

Distribution: destination nodes sharded across 8 cores (6250 each, padded
to 6272 = 49 tiles of 128). mlp_h computed per-shard then AllGathered as a
pair-padded table. Per-edge mlp_h values and selected feat rows are fetched
with dma_gather (int16 indices => pair-of-rows tables of 25000 entries).
"""

import os
import numpy as np
import ml_dtypes

import concourse.bass as bass
import concourse.mybir as mybir
import concourse.tile as tile
import concourse.bacc as bacc
from concourse.library_config import mlp as mlp_lib
from concourse.bass_utils import run_bass_kernel_spmd
from concourse.masks import make_identity

F32 = mybir.dt.float32
BF16 = mybir.dt.bfloat16
I16 = mybir.dt.int16

N = 50000
D = 128
K = 32
SEL = 16
CORES = 8
NS = N // CORES            # 6250 dst nodes per core
T = 49                     # tiles of 128 (6272 padded)
NT = T * 128
DO = 64
W = N // 2                 # 25000 pair blocks
E3 = 3
NIU = E3 * K * 128         # u-gather indices per tile = 12288
NIF = E3 * SEL * 128       # feat-gather indices per tile = 6144


def split_excess_waits(nc, max_waits=1):
    for fn in nc.m.functions:
        for bb in list(fn.blocks):
            out = []
            for ins in bb.instructions:
                si = ins.sync_info
                if si is not None and si.on_wait and len(si.on_wait) > max_waits:
                    waits = list(si.on_wait)
                    extra = waits[:-max_waits]
                    for i in range(0, len(extra), max_waits):
                        nop = nc.engines[ins.engine].nop(nofuse=True).ins
                        popped = False
                        for b2 in fn.blocks:
                            if b2.instructions and b2.instructions[-1] is nop:
                                b2.instructions.pop()
                                popped = True
                                break
                        assert popped
                        nop.sync_info = mybir.SyncInfo(
                            on_wait=extra[i : i + max_waits], on_update=[]
                        )
                        out.append(nop)
                    si.on_wait = waits[-max_waits:]
                out.append(ins)
            bb.instructions[:] = out


def ap_(t_ap, off, dims):
    """Manual AP: keep the tile's partition dim, set free dims [[step, count],...]."""
    return bass.AP(t_ap.tensor, t_ap.offset + off, [list(t_ap.ap[0])] + [list(d) for d in dims])


def build_nc(repeat=1, stage=4):
    nc = bacc.Bacc("TRN2", target_bir_lowering=False, debug=False, num_devices=CORES, num_swdge_queues=4)

    feat_pair = nc.dram_tensor("feat_pair", [W, 256], BF16, kind="ExternalInput")
    feat_dst = nc.dram_tensor("feat_dst", [NT, D], F32, kind="ExternalInput")
    wmlp_in = nc.dram_tensor("wmlp", [D, 2], F32, kind="ExternalInput")
    bmlp_in = nc.dram_tensor("bmlp", [128, 2], F32, kind="ExternalInput")
    wlin_in = nc.dram_tensor("wlin", [D, DO], F32, kind="ExternalInput")
    blin_in = nc.dram_tensor("blin", [128, DO], F32, kind="ExternalInput")
    idxu_in = nc.dram_tensor("idxu", [T, 128, NIU // 16], I16, kind="ExternalInput")
    kb_in = nc.dram_tensor("keybase", [T, 128, E3 * K], F32, kind="ExternalInput")
    selq_in = nc.dram_tensor("selq", [8, 128, 128], F32, kind="ExternalInput")
    out_d = nc.dram_tensor("out", [NT, DO], F32, kind="ExternalOutput")

    with tile.TileContext(nc) as tc:
        with (
            tc.tile_pool(name="const", bufs=1) as cp,
            tc.tile_pool(name="sb", bufs=3) as sb,
            tc.tile_pool(name="big", bufs=2) as bigp,
            tc.tile_pool(name="ps", bufs=2, space="PSUM") as ps,
            tc.tile_pool(name="psq", bufs=2, space="PSUM") as psq,
            tc.tile_pool(name="dram", bufs=1, space="DRAM") as dp,
        ):
            nc.gpsimd.load_library(mlp_lib)

            ident = cp.tile([128, 128], F32)
            make_identity(nc, ident[:])
            wmlp_sb = cp.tile([128, 2], F32)
            nc.sync.dma_start(wmlp_sb[:], wmlp_in[:])
            bmlp_sb = cp.tile([128, 2], F32)
            nc.sync.dma_start(bmlp_sb[:], bmlp_in[:])
            wlin_sb = cp.tile([128, DO], F32)
            nc.sync.dma_start(wlin_sb[:], wlin_in[:])
            blin_sb = cp.tile([128, DO], F32)
            nc.sync.dma_start(blin_sb[:], blin_in[:])
            selq_sb = cp.tile([128, 8 * 128], F32)
            nc.sync.dma_start(
                selq_sb[:].rearrange("p (q c) -> p q c", q=8),
                selq_in[:].rearrange("q p c -> p q c"),
            )

            feat_sb = cp.tile([128, T * D], F32)
            fd3 = feat_dst[:].rearrange("(t p) d -> p t d", p=128)
            fs3 = feat_sb[:].rearrange("p (t d) -> p t d", t=T)
            nc.sync.dma_start(fs3[:, :25, :], fd3[:, :25, :])
            nc.sync.dma_start(fs3[:, 25:, :], fd3[:, 25:, :])

            # ---------- phase 0: mlp_h for own shard ----------
            mh_pre = cp.tile([128, 2 * T], F32)
            for t in range(T):
                ft = ap_(feat_sb[:], t * D, [[1, D]])
                psF = ps.tile([128, 128], F32, tag="ptr")
                nc.tensor.transpose(psF[:], ft, ident[:])
                ftT = sb.tile([128, 128], F32, tag="ftT")
                nc.vector.tensor_copy(ftT[:], psF[:])
                pre = psq.tile([128, 2], F32, tag="pre")
                nc.tensor.matmul(pre[:], lhsT=ftT[:], rhs=wmlp_sb[:], start=True, stop=True)
                nc.vector.tensor_tensor(
                    out=ap_(mh_pre[:], 2 * t, [[1, 2]]),
                    in0=pre[:],
                    in1=bmlp_sb[:],
                    op=mybir.AluOpType.add,
                )
            mh = cp.tile([128, 2 * T], F32)
            nc.scalar.activation(mh[:], mh_pre[:], mybir.ActivationFunctionType.Tanh)
            mhneg = cp.tile([128, 2 * T], F32)
            nc.vector.tensor_scalar(mhneg[:], mh[:], -1.0, None, mybir.AluOpType.mult)

            # write pair-padded u slab: u_slab[64*t + s, lane*2 + c] = mh[2*s+lane, 2*t+c]
            u_slab = dp.tile([NS // 2, 64], F32)
            u_pad = dp.tile([W, 64], F32)
            for lane in range(2):
                # tiles 0..47 (full)
                nc.sync.dma_start(
                    out=u_slab[: 48 * 64, lane * 2 : lane * 2 + 2].rearrange(
                        "(t s) c -> s t c", s=64
                    ),
                    in_=mh[lane:128:2, : 2 * 48].rearrange("p (t c) -> p t c", c=2),
                )
                # tile 48 partial: 106 nodes -> 53 pairs
                nc.sync.dma_start(
                    out=u_slab[48 * 64 : 48 * 64 + 53, lane * 2 : lane * 2 + 2].rearrange(
                        "(t s) c -> s t c", s=53
                    ),
                    in_=mh[lane:106:2, 2 * 48 : 2 * 49].rearrange("p (t c) -> p t c", c=2),
                )
            nc.gpsimd.collective_compute(
                "AllGather",
                mybir.AluOpType.bypass,
                replica_groups=[list(range(CORES))],
                ins=[u_slab.opt()],
                outs=[u_pad.opt()],
            )

            # ---------- phase 1 ----------
            qrr = [0]
            for t in [tt for _ in range(repeat) for tt in range(T)]:
                idxu_t = sb.tile([128, NIU // 16], I16, tag="idxu")
                nc.sync.dma_start(idxu_t[:], idxu_in[t])
                kb_t = sb.tile([128, E3 * K], F32, tag="kb")
                nc.sync.dma_start(kb_t[:], kb_in[t])

                gu = bigp.tile([128, E3 * K * 64], F32, tag="gu")
                for j in range(4):
                    nc.gpsimd.dma_gather(
                        ap_(gu[:], j * 24 * 64, [[64, 24], [1, 64]]),
                        u_pad[:],
                        ap_(idxu_t[:], j * 192, [[1, 192]]),
                        NIU // 4,
                        NIU // 4,
                        64,
                        single_packet=False,
                        queue_num=qrr[0] % 4,
                    )
                    qrr[0] += 1

                if stage < 2:
                    continue
                m0neg = ap_(mhneg[:], 2 * t, [[1, 1]])
                m1neg = ap_(mhneg[:], 2 * t + 1, [[1, 1]])
                guc = lambda j: ap_(gu[:], j, [[64, E3 * K]])
                t0 = sb.tile([128, E3 * K], F32, tag="t0")
                t1 = sb.tile([128, E3 * K], F32, tag="t1")
                de = sb.tile([128, E3 * K], F32, tag="de")
                do_ = sb.tile([128, E3 * K], F32, tag="do")
                AF = mybir.ActivationFunctionType
                nc.scalar.activation(t0[:], guc(0), AF.Abs, bias=m0neg)
                nc.scalar.activation(t1[:], guc(1), AF.Abs, bias=m1neg)
                nc.vector.tensor_tensor(out=de[:], in0=t0[:], in1=t1[:], op=mybir.AluOpType.add)
                nc.scalar.activation(t0[:], guc(2), AF.Abs, bias=m0neg)
                nc.scalar.activation(t1[:], guc(3), AF.Abs, bias=m1neg)
                nc.vector.tensor_tensor(out=do_[:], in0=t0[:], in1=t1[:], op=mybir.AluOpType.add)
                # parity select: par = kb mod 2  (keybase = (31-k)*65536 + v + 1; v parity flips the +1)
                # NOTE: keybase parity = (v+1) mod 2 = 1 - v%2 -> par_v = 1 - (kb mod 2)
                kbi = sb.tile([128, E3 * K], mybir.dt.int32, tag="kbi")
                nc.vector.tensor_copy(kbi[:], kb_t[:])
                kpar = sb.tile([128, E3 * K], mybir.dt.int32, tag="kpar")
                nc.vector.tensor_scalar(kpar[:], kbi[:], 1, None, mybir.AluOpType.bitwise_and)
                # d = select(v odd -> do_, else de). v odd <=> kpar == 0
                dd = sb.tile([128, E3 * K], F32, tag="dd")
                nc.vector.tensor_copy(dd[:], do_[:])
                nc.vector.copy_predicated(dd[:], kpar[:], de[:])  # kpar!=0 -> v even -> de
                # dp = 5 - d  (in (1, 5]); topk-16 per etype
                dpv = sb.tile([128, E3 * K], F32, tag="dpv")
                nc.vector.tensor_scalar(dpv[:], dd[:], -1.0, 5.0, mybir.AluOpType.mult, mybir.AluOpType.add)
                w1 = sb.tile([128, E3 * K], F32, tag="w1")
                w2 = sb.tile([128, E3 * K], F32, tag="w2")
                for e in range(E3):
                    dpe = ap_(dpv[:], K * e, [[1, K]])
                    w1e = ap_(w1[:], K * e, [[1, K]])
                    w2e = ap_(w2[:], K * e, [[1, K]])
                    mx = sb.tile([128, 8], F32, tag="mx")
                    nc.vector.max(mx[:], dpe)
                    nc.vector.match_replace(w1e, in_to_replace=mx[:], in_values=dpe, imm_value=0.0)
                    mx2 = sb.tile([128, 8], F32, tag="mx2")
                    nc.vector.max(mx2[:], w1e)
                    nc.vector.match_replace(w2e, in_to_replace=mx2[:], in_values=w1e, imm_value=0.0)
                mask = sb.tile([128, E3 * K], F32, tag="mask")
                nc.vector.tensor_scalar(mask[:], w2[:], 0.0, None, mybir.AluOpType.is_equal)
                key = sb.tile([128, E3 * K], F32, tag="key")
                nc.vector.tensor_tensor(out=key[:], in0=mask[:], in1=kb_t[:], op=mybir.AluOpType.mult)
                # extract the 16 selected keys per etype
                sk = sb.tile([128, E3 * SEL], F32, tag="sk")
                kz = sb.tile([128, E3 * K], F32, tag="kz")
                for e in range(E3):
                    keye = ap_(key[:], K * e, [[1, K]])
                    kze = ap_(kz[:], K * e, [[1, K]])
                    sk1 = ap_(sk[:], SEL * e, [[1, 8]])
                    sk2 = ap_(sk[:], SEL * e + 8, [[1, 8]])
                    mxa = sb.tile([128, 8], F32, tag="mxa")
                    nc.vector.max(mxa[:], keye)
                    nc.vector.tensor_copy(sk1, mxa[:])
                    nc.vector.match_replace(kze, in_to_replace=mxa[:], in_values=keye, imm_value=0.0)
                    mxb = sb.tile([128, 8], F32, tag="mxb")
                    nc.vector.max(mxb[:], kze)
                    nc.vector.tensor_copy(sk2, mxb[:])
                # vsel = (key-1) mod 65536 ; psel = vsel mod 2 ; wsel = (vsel-psel)/2
                vt = sb.tile([128, E3 * SEL], F32, tag="vt")
                nc.vector.tensor_scalar(vt[:], sk[:], 1.0, None, mybir.AluOpType.subtract)
                vti = sb.tile([128, E3 * SEL], mybir.dt.int32, tag="vti")
                nc.vector.tensor_copy(vti[:], vt[:])
                vi = sb.tile([128, E3 * SEL], mybir.dt.int32, tag="vi")
                nc.vector.tensor_scalar(vi[:], vti[:], 65535, None, mybir.AluOpType.bitwise_and)
                vsel = sb.tile([128, E3 * SEL], F32, tag="vsel")
                nc.vector.tensor_copy(vsel[:], vi[:])
                pi = sb.tile([128, E3 * SEL], mybir.dt.int32, tag="pi")
                nc.vector.tensor_scalar(pi[:], vi[:], 1, None, mybir.AluOpType.bitwise_and)
                psel = sb.tile([128, E3 * SEL], F32, tag="psel")
                nc.vector.tensor_copy(psel[:], pi[:])
                wsel = sb.tile([128, E3 * SEL], F32, tag="wsel")
                nc.vector.tensor_tensor(out=wsel[:], in0=vsel[:], in1=psel[:], op=mybir.AluOpType.subtract)
                nc.vector.tensor_scalar(wsel[:], wsel[:], 0.5, None, mybir.AluOpType.mult)
                if stage < 3:
                    continue
                # fold to wrapped idx layout: widx[p, (e,j,q)] = wsel[q*16 + p%16, (e,j)]
                widx = sb.tile([128, NIF // 16], I16, tag="widx")
                for q in range(8):
                    pq = psq.tile([128, E3 * SEL], F32, tag="pq")
                    nc.tensor.matmul(
                        pq[:],
                        lhsT=ap_(selq_sb[:], q * 128, [[1, 128]]),
                        rhs=wsel[:],
                        start=True,
                        stop=True,
                    )
                    nc.vector.tensor_copy(ap_(widx[:], q, [[8, E3 * SEL]]), pq[:])
                gf = bigp.tile([128, E3 * SEL * 256], BF16, tag="gf")
                for j in range(2):
                    nc.gpsimd.dma_gather(
                        ap_(gf[:], j * 24 * 256, [[256, 24], [1, 256]]),
                        feat_pair[:],
                        ap_(widx[:], j * 192, [[1, 192]]),
                        NIF // 2,
                        NIF // 2,
                        256,
                        single_packet=False,
                        queue_num=qrr[0] % 4,
                    )
                    qrr[0] += 1
                if stage < 4:
                    continue
                # parity-weighted mean over the 16 selected
                wm = sb.tile([128, E3 * SEL * 2], F32, tag="wm")
                nc.vector.tensor_copy(ap_(wm[:], 1, [[2, E3 * SEL]]), psel[:])
                nc.vector.tensor_scalar(
                    ap_(wm[:], 0, [[2, E3 * SEL]]), psel[:], -1.0, 1.0,
                    mybir.AluOpType.mult, mybir.AluOpType.add,
                )
                nc.vector.tensor_tensor(
                    out=ap_(gf[:], 0, [[256, E3 * SEL], [128, 2], [1, 128]]),
                    in0=ap_(gf[:], 0, [[256, E3 * SEL], [128, 2], [1, 128]]),
                    in1=ap_(wm[:], 0, [[2, E3 * SEL], [1, 2], [0, 128]]),
                    op=mybir.AluOpType.mult,
                )
                agg = sb.tile([128, E3 * D], F32, tag="agg")
                nc.vector.tensor_reduce(
                    agg[:].rearrange("p (e f) -> p e f", e=E3),
                    ap_(gf[:], 0, [[SEL * 256, E3], [1, 128], [256, SEL], [128, 2]]),
                    axis=mybir.AxisListType.XY,
                    op=mybir.AluOpType.add,
                )
                aggt = sb.tile([128, E3 * D], F32, tag="aggt")
                nc.scalar.activation(
                    aggt[:], agg[:], mybir.ActivationFunctionType.Tanh, scale=1.0 / SEL
                )
                s3 = sb.tile([128, D], F32, tag="s3")
                nc.vector.tensor_reduce(
                    s3[:].rearrange("p (f o) -> p f o", o=1),
                    ap_(aggt[:], 0, [[1, D], [D, E3]]),
                    axis=mybir.AxisListType.X,
                    op=mybir.AluOpType.add,
                )
                h1 = sb.tile([128, D], F32, tag="h1")
                nc.vector.tensor_scalar(h1[:], s3[:], 0.5, None, mybir.AluOpType.mult)
                nc.vector.tensor_tensor(
                    out=h1[:], in0=h1[:], in1=ap_(feat_sb[:], t * D, [[1, D]]),
                    op=mybir.AluOpType.add,
                )
                h2 = sb.tile([128, D], F32, tag="h2")
                nc.scalar.activation(h2[:], h1[:], mybir.ActivationFunctionType.Tanh)
                psH = ps.tile([128, 128], F32, tag="ptr")
                nc.tensor.transpose(psH[:], h2[:], ident[:])
                h2T = sb.tile([128, 128], F32, tag="h2T")
                nc.vector.tensor_copy(h2T[:], psH[:])
                og = psq.tile([128, DO], F32, tag="og")
                nc.tensor.matmul(og[:], lhsT=h2T[:], rhs=wlin_sb[:], start=True, stop=True)
                osb = sb.tile([128, DO], F32, tag="osb")
                nc.vector.tensor_tensor(out=osb[:], in0=og[:], in1=blin_sb[:], op=mybir.AluOpType.add)
                nc.sync.dma_start(out_d[t * 128 : (t + 1) * 128, :], osb[:])

    nc.compile()
    split_excess_waits(nc)
    return nc


_CACHE = {}


def _host_prep(feat, W_mlp, b_mlp, W_lin, b_lin, nbr0, nbr1, nbr2):
    feat = np.asarray(feat, dtype=np.float32)
    nbrs = [np.asarray(x, dtype=np.int32) for x in (nbr0, nbr1, nbr2)]
    feat_pair = np.ascontiguousarray(
        feat.astype(ml_dtypes.bfloat16).reshape(W, 256)
    )
    selq = np.zeros((8, 128, 128), np.float32)
    p2 = np.arange(128)
    for q in range(8):
        selq[q, q * 16 + (p2 % 16), p2] = 1.0

    wmlp = np.ascontiguousarray(np.asarray(W_mlp, np.float32))
    bmlp = np.broadcast_to(np.asarray(b_mlp, np.float32), (128, 2)).copy()
    wlin = np.ascontiguousarray(np.asarray(W_lin, np.float32))
    blin = np.broadcast_to(np.asarray(b_lin, np.float32), (128, DO)).copy()

    in_maps = []
    for c in range(CORES):
        sl = slice(c * NS, (c + 1) * NS)
        fd = np.zeros((NT, D), np.float32)
        fd[:NS] = feat[sl]
        nb = np.zeros((E3, NT, K), np.int32)
        for e in range(E3):
            nb[e, :NS] = nbrs[e][sl]
        # u-gather wrapped idx: [T, 16->128, 768]
        v_t = nb.reshape(E3, T, 8, 16, K)  # [e, t, q, r, k]
        idxu = (v_t >> 1).astype(np.int16).transpose(1, 3, 0, 4, 2)  # [t, r, e, k, q]
        idxu = idxu.reshape(T, 16, NIU // 16)
        idxu = np.ascontiguousarray(np.tile(idxu, (1, 8, 1)))
        # keybase [T, 128, 96]
        kk = np.arange(K)[None, None, :]
        kb = ((31 - kk) * 65536 + nb + 1).astype(np.float32)  # [e, NT, K]
        kb = kb.reshape(E3, T, 128, K).transpose(1, 2, 0, 3).reshape(T, 128, E3 * K)
        in_maps.append(
            {
                "feat_pair": feat_pair,
                "feat_dst": fd,
                "wmlp": wmlp,
                "bmlp": bmlp,
                "wlin": wlin,
                "blin": blin,
                "idxu": idxu,
                "keybase": np.ascontiguousarray(kb),
                "selq": selq,
            }
        )
    return in_maps


def bench(feat, W_mlp, b_mlp, W_lin, b_lin, nbr0, nbr1, nbr2, iters=5):
    """Timed repeated execution with device-resident inputs. Returns (ns, out)."""
    import time
    import jax
    from jax.sharding import Mesh, PartitionSpec, NamedSharding
    from jax.experimental.shard_map import shard_map
    from concourse import bass2jax

    if "nc" not in _CACHE:
        _CACHE["nc"] = build_nc()
    nc = _CACHE["nc"]
    in_maps = _host_prep(feat, W_mlp, b_mlp, W_lin, b_lin, nbr0, nbr1, nbr2)

    bass2jax.install_neuronx_cc_hook()
    partition_name = nc.partition_id_tensor.name if nc.partition_id_tensor else None
    import concourse.mybir as mybir_
    in_names, out_names, out_avals, zero_outs = [], [], [], []
    for alloc in nc.m.functions[0].allocations:
        if not isinstance(alloc, mybir_.MemoryLocationSet):
            continue
        name = alloc.memorylocations[0].name
        if alloc.kind == "ExternalInput":
            if name != partition_name:
                in_names.append(name)
        elif alloc.kind == "ExternalOutput":
            shape = tuple(alloc.tensor_shape)
            dtype = mybir_.dt.np(alloc.dtype)
            out_names.append(name)
            out_avals.append(jax.core.ShapedArray(shape, dtype))
            zero_outs.append(np.zeros(shape, dtype))
    n_params = len(in_names)
    n_outs = len(out_avals)
    all_in_names = list(in_names) + list(out_names)
    if partition_name is not None:
        all_in_names.append(partition_name)

    def _body(*args):
        operands = list(args)
        if partition_name is not None:
            operands.append(bass2jax.partition_id_tensor())
        outs = bass2jax._bass_exec_p.bind(
            *operands,
            out_avals=tuple(out_avals),
            in_names=tuple(all_in_names),
            out_names=tuple(out_names),
            lowering_input_output_aliases=(),
            sim_require_finite=True,
            sim_require_nnan=True,
            nc=nc,
        )
        return tuple(outs)

    devices = jax.devices()[:CORES]
    mesh = Mesh(np.asarray(devices), ("core",))
    spec = PartitionSpec("core")
    sharded = jax.jit(
        shard_map(_body, mesh=mesh, in_specs=(spec,) * (n_params + n_outs),
                  out_specs=(spec,) * n_outs, check_rep=False),
        keep_unused=True,
    )
    sh = NamedSharding(mesh, spec)
    dev_in = [
        jax.device_put(
            np.concatenate([np.asarray(in_maps[c][nm]) for c in range(CORES)], axis=0), sh
        )
        for nm in in_names
    ]
    dev_zeros = [
        jax.device_put(np.zeros((CORES * z.shape[0], *z.shape[1:]), z.dtype), sh)
        for z in zero_outs
    ]
    out = sharded(*dev_in, *dev_zeros)
    jax.block_until_ready(out)
    times = []
    for _ in range(iters):
        t0 = time.perf_counter()
        out = sharded(*dev_in, *dev_zeros)
        jax.block_until_ready(out)
        times.append(time.perf_counter() - t0)
    ns = int(min(times) * 1e9)
    res = np.asarray(out[out_names.index("out")]).reshape(CORES, NT, DO)
    full = np.concatenate([res[c][:NS] for c in range(CORES)], axis=0).astype(np.float32)
    return ns, full, times


def kernel(feat, W_mlp, b_mlp, W_lin, b_lin, nbr0, nbr1, nbr2):
    if "nc" not in _CACHE:
        _CACHE["nc"] = build_nc()
    nc = _CACHE["nc"]
    in_maps = _host_prep(feat, W_mlp, b_mlp, W_lin, b_lin, nbr0, nbr1, nbr2)
    trace = bool(os.environ.get("BASS_KERNEL_PROFILE"))
    res = run_bass_kernel_spmd(nc, in_maps, list(range(CORES)), trace=trace)
    if trace:
        _CACHE["last_exec_ns"] = res.exec_time_ns
    out = np.concatenate([res.results[c]["out"][:NS] for c in range(CORES)], axis=0)
    return out.astype(np.float32)

